# revision 51
# baseline (speedup 1.0000x reference)
"""Trainium2 Bass kernel for temporal-decay causal multi-head attention.

Problem: nn_MultiHeadAttention_9053791060340
  B=4, S=2048, DM=512, H=8, HD=64.
  out = softmax((Q K^T / sqrt(HD)) * exp(-rate*|t_i - t_j|) with causal mask) V,
  then out-projection.

Sharding: 8 cores = 4 batches x 2 head-groups (4 heads each). Each core
computes a partial out-projection [S, DM] for its head group; the host sums
the two partials per batch and adds the output bias.

Device algorithm (per core); matmul inputs in bf16 with fp32 PSUM
accumulation:
  - scores are computed TRANSPOSED (S^T[k, q] = k . q) with a no-max softmax
    (scores here are bounded so exp never overflows in fp32); the denominator
    comes free from a ones-column appended to V.
  - temporal decay factorizes on sorted days: exp(-r(t_i - t_k)) = a_i * b_k
    with a per-q-group reference t0. a (and 1/sqrt(HD)) folds into q^T; b
    folds into a per-q-group scaled copy of k^T.
  - pairs far enough apart that |s|*decay < ~3e-4 contribute weight
    exp(s*decay) = 1 + O(3e-4); all k-chunks entirely below the cutoff
    collapse into a rank-1 update ctx^T += PrefVsum x ones with the count
    added to the denominator (prefix sums computed on the host).
  - causal masking: the diagonal 128-block of exp(scores) is multiplied by
    a 0/1 tril tile on the gpsimd (SBUF side, post-exp); blocks fully above
    the diagonal are skipped via the streamed q-range.
  - softmax division: the two denominator PSUM rows land on partitions 0/64
    of a [65, QG] tile pre-filled with ones, one reciprocal_approx_fast
    covers both at full partition parallelism, and a single fp32 matmul
    against a [65, 128] selector broadcasts the rec rows across partitions
    -- no DMA, no gpsimd, no transposes in the division chain.
"""

import os

import ml_dtypes
import numpy as np

import concourse.tile as tile
from concourse import bacc
from concourse import mybir
from concourse.bass_utils import run_bass_kernel_spmd

F32 = mybir.dt.float32

B, S, DM, H = 4, 2048, 512, 8
HD = DM // H          # 64
NCORES = 8
NHG = 2               # head groups == cores per batch
NH = H // NHG         # heads per core
HGD = NH * HD         # 256 output dims per core
QG = 512              # q-group width
NQG = S // QG         # 4
KC = 128              # k chunk (partition dim of S^T)
NKC = S // KC         # 16
P = 128
NEG = -1.0e30

_cache: dict = {}


# --------------------------------------------------------------------------
# device program
# --------------------------------------------------------------------------

def _build_fast(kc_lo: tuple, wmax: int, with_bqk: bool, with_bv: bool,
                use_bf16: bool = True):
    """Build the SPMD Bass program.

    kc_lo[qg] = first near k-chunk per q-group (static across cores; chunks
    below it are covered by the prefix-sum rank-1 update). wmax = max
    near-window width in elements for the scaled-k tile.
    """
    nc = bacc.Bacc()
    MDT = mybir.dt.bfloat16 if use_bf16 else F32
    KO = DM // P  # 4 k-sub-chunks for DM-contraction

    # x / weights arrive pre-rearranged to the device layout so every DMA
    # line is per-partition contiguous (large descriptors, full queue rate)
    xT_d = nc.declare_dram_parameter("xT", [P, KO * S], MDT, False)
    wq_d = nc.declare_dram_parameter("wqT", [P, KO * HGD], MDT, False)
    wk_d = nc.declare_dram_parameter("wkT", [P, KO * HGD], MDT, False)
    wv_d = nc.declare_dram_parameter("wvT", [P, KO * HGD], MDT, False)
    wo_d = nc.declare_dram_parameter("woT", [P, 2 * DM], MDT, False)
    av_d = nc.declare_dram_parameter("avec", [1, S], F32, False)
    bv_d = nc.declare_dram_parameter("bvec", [NQG, S], F32, False)
    # packed f32 consts: cols 0-127 sel65 (f32r bits), 128-143 pft,
    # 144-159 cnt (row 0), 160-287 additive causal band mask
    cp_d = nc.declare_dram_parameter("cpk", [P, 288], F32, False)
    s65_d = nc.declare_dram_parameter("sel65r", [P, P], mybir.dt.float32r,
                                      False)
    if with_bqk:
        bq_d = nc.declare_dram_parameter("bq", [P, 2], F32, False)
        bk_d = nc.declare_dram_parameter("bk", [P, 2], F32, False)
    if with_bv:
        bvb_d = nc.declare_dram_parameter("bvb", [1, HGD], F32, False)
    out_d = nc.declare_dram_parameter("outp", [S, DM], F32, True)

    VW = HD + 1   # 65: V columns plus ones column

    with tile.TileContext(nc) as tc:
        with (
            tc.tile_pool(name="const", bufs=1) as const,
            tc.tile_pool(name="ppool", bufs=2, space="PSUM") as ppool,
            tc.tile_pool(name="spool", bufs=2, space="PSUM") as spool,
            tc.tile_pool(name="cpool", bufs=2, space="PSUM") as cpool,
            tc.tile_pool(name="ptp", bufs=3) as ptp,
            tc.tile_pool(name="ktsp", bufs=3) as ktsp,
            tc.tile_pool(name="bvqp", bufs=3) as bvqp,
            tc.tile_pool(name="ctxp", bufs=6) as ctxp,
            tc.tile_pool(name="densp", bufs=3) as densp,
        ):
            # ---- constant loads. The first wave is issued from FOUR
            # different sequencers in parallel (each dma_start costs ~600ns
            # of issue time on its engine; serializing them all on Sync is
            # what made the old startup 8us+) ----
            wq_sb = const.tile([P, KO, HGD], MDT)
            wq_r = wq_d[:].rearrange("p (ko m) -> p ko m", m=HGD)
            nc.sync.dma_start(wq_sb, wq_r)
            xT_sb = const.tile([P, KO, S], MDT)
            xT_r = xT_d[:].rearrange("p (ko s) -> p ko s", s=S)
            wk_sb = const.tile([P, KO, HGD], MDT)
            nc.gpsimd.dma_start(wk_sb,
                                wk_d[:].rearrange("p (ko m) -> p ko m", m=HGD))
            nc.scalar.dma_start(xT_sb[:, 0, 0:QG], xT_r[:, 0, 0:QG])
            nc.sync.dma_start(xT_sb[:, 1, 0:QG], xT_r[:, 1, 0:QG])
            nc.gpsimd.dma_start(xT_sb[:, 2, 0:QG], xT_r[:, 2, 0:QG])
            nc.scalar.dma_start(xT_sb[:, 3, 0:QG], xT_r[:, 3, 0:QG])
            wv_sb = const.tile([P, KO, HGD], MDT)
            nc.gpsimd.dma_start(wv_sb,
                                wv_d[:].rearrange("p (ko m) -> p ko m", m=HGD))
            avec_full = const.tile([P, S], F32)
            nc.gpsimd.dma_start(avec_full[:, 0:QG],
                                av_d[:][:, 0:QG].to_broadcast([P, QG]))
            cpk_sb = const.tile([P, 288], F32)
            sel65r_sb = const.tile([P, P], mybir.dt.float32r)
            pft_sb = cpk_sb[0:HD, P:P + 16]
            cnt_sb = cpk_sb[0:1, P + 16:P + 32]
            bm_sb = cpk_sb[:, P + 32:P + 160]
            nc.sync.dma_start(xT_sb[:, :, QG:2 * QG], xT_r[:, :, QG:2 * QG])
            nc.sync.dma_start(cpk_sb, cp_d[:])
            for ns in range(2, 4):
                nc.sync.dma_start(xT_sb[:, :, ns * QG:(ns + 1) * QG],
                                  xT_r[:, :, ns * QG:(ns + 1) * QG])
            nc.sync.dma_start(sel65r_sb, s65_d[:])
            # head-pair on partitions: rows 0-63 = even head, 64-127 = odd
            wo_sb = const.tile([P, 2, DM], MDT)
            nc.sync.dma_start(wo_sb,
                              wo_d[:].rearrange("p (hp n) -> p hp n", n=DM))
            for ns in range(1, 4):
                nc.sync.dma_start(
                    avec_full[:, ns * QG:(ns + 1) * QG],
                    av_d[:][:, ns * QG:(ns + 1) * QG].to_broadcast([P, QG]))
            if with_bqk:
                bq_sb = const.tile([P, 2], F32)
                nc.sync.dma_start(bq_sb, bq_d[:])
                bk_sb = const.tile([P, 2], F32)
                nc.sync.dma_start(bk_sb, bk_d[:])
            if with_bv:
                bv_full = const.tile([P, HGD], F32)
                nc.sync.dma_start(bv_full, bvb_d[:].to_broadcast([P, HGD]))

            # PE warm-up: the DVFS p-state needs ~3us of continuous busy
            # to reach full clock. The PE would otherwise idle until the
            # first weight/x DMAs land (~10.5us) and then ramp through the
            # opening projections. Dummy matmuls on a memset tile (results
            # never read) keep it busy from the end of the preamble.
            warm_sb = const.tile([P, QG], MDT, tag="warm")
            nc.vector.memset(warm_sb, 1.0)
            for _ in range(8):
                wps = ppool.tile([P, QG], F32, tag="pp")
                nc.tensor.matmul(wps, lhsT=warm_sb[:, 0:P], rhs=warm_sb,
                                 start=True, stop=True)

            kT_sb = const.tile([P, 2, S], F32)
            qT_sb = const.tile([P, 2, S], MDT, name='qT_cast')
            va_sb = const.tile([P, NKC, NH * VW], MDT)
            va_resh = va_sb.rearrange("p s (h c) -> p s h c", c=VW)
            nc.vector.memset(va_resh[:, :, :, HD], 1.0)

            def proj_qk(ns):
                """q/k projections for sequence slice ns (one q-group).
                The q PSUM tiles are returned undrained: the a-fold consumes
                them straight from PSUM (no intermediate copy)."""
                sl = slice(ns * QG, (ns + 1) * QG)
                qps = []
                for w_sb, b_sb in ((wq_sb, "q"), (wk_sb, "k")):
                    for mc in range(2):
                        ps = ppool.tile([P, QG], F32, tag="pp")
                        for ki in range(KO):
                            nc.tensor.matmul(
                                ps,
                                lhsT=w_sb[:, ki, mc * P:(mc + 1) * P],
                                rhs=xT_sb[:, ki, sl],
                                start=(ki == 0),
                                stop=(ki == KO - 1),
                            )
                        if b_sb == "q":
                            if with_bqk:
                                qf = ptp.tile([P, QG], F32, tag="qbias")
                                with tc.high_priority(offset=-200):
                                    nc.scalar.activation(
                                        qf, ps,
                                        mybir.ActivationFunctionType.Identity,
                                        bias=bq_sb[:, mc:mc + 1],
                                    )
                                qps.append(qf)
                            else:
                                qps.append(ps)
                        else:
                            # lightly deprioritized: the NEXT group's scores
                            # wait on these via the b-fold
                            with tc.high_priority(offset=-200):
                                if with_bqk:
                                    nc.scalar.activation(
                                        kT_sb[:, mc, sl], ps,
                                        mybir.ActivationFunctionType.Identity,
                                        bias=bk_sb[:, mc:mc + 1],
                                    )
                                else:
                                    nc.vector.tensor_copy(
                                        kT_sb[:, mc, sl], ps)
                return qps

            def fold(ns, kts, kw, klo, qps):
                """fold a into q^T (DVE, straight from the q PSUM) and b
                into k^T (gpsimd), mc0 first so the first score matmuls of
                the group unblock early."""
                sl = slice(ns * QG, (ns + 1) * QG)
                for mc in range(2):
                    nc.vector.tensor_tensor(
                        qT_sb[:, mc, sl], qps[mc],
                        avec_full[:, sl], mybir.AluOpType.mult,
                    )
                    nc.gpsimd.tensor_tensor(
                        kts[:, mc, :kw], kT_sb[:, mc, klo:klo + kw],
                        bvf_cur[ns][:, :kw], mybir.AluOpType.mult,
                    )

            def proj_v(ns):
                sl = slice(ns * QG, (ns + 1) * QG)
                for sc in range(4 * ns, 4 * ns + 4):
                    ps = ppool.tile([P, HGD], F32, tag="pp")
                    for ki in range(KO):
                        nc.tensor.matmul(
                            ps,
                            lhsT=xT_sb[:, ki, sc * P:(sc + 1) * P],
                            rhs=wv_sb[:, ki, :],
                            start=(ki == 0),
                            stop=(ki == KO - 1),
                        )
                    with tc.high_priority(offset=-600):
                        # one strided copy per sc covers all 4 heads
                        dst = va_resh[:, sc, :, 0:HD]
                        src = ps[:].rearrange("p (h c) -> p h c", c=HD)
                        if with_bv:
                            nc.vector.tensor_tensor(
                                dst, src,
                                bv_full[:].rearrange("p (h c) -> p h c", c=HD),
                                mybir.AluOpType.add,
                            )
                        else:
                            nc.vector.tensor_copy(dst, src)

            bvf_cur = {}

            def prep(qg):
                """b-vector broadcast DMA for group qg; returns kts tile."""
                klo = kc_lo[qg] * KC
                kw = (qg + 1) * QG - klo
                bvf = bvqp.tile([P, wmax], F32, tag="bvf")
                nc.gpsimd.dma_start(
                    bvf[:, :kw],
                    bv_d[:][qg:qg + 1, klo:khi_e(qg)].to_broadcast([P, kw]),
                )
                bvf_cur[qg] = bvf
                kts = ktsp.tile([P, 2, wmax], MDT, tag="kts")
                return kts, kw, klo

            def khi_e(qg):
                return (qg + 1) * QG

            # dens65 tiles: den_h0 on partition 0, den_h1 on partition 64.
            # Rows 1-63 / 65+ are set to 1.0 ONCE; reciprocal_approx_fast
            # runs over all 65 partitions (1/1=1 in unused rows -- keeps the
            # undefined-at-zero edge case away) and a single fp32 matmul
            # against sel65 broadcasts the two rec rows across partitions.
            dens65s = []
            for i in range(3):
                t65 = const.tile([VW, QG], F32, tag=f"d65_{i}")
                nc.vector.memset(t65, 1.0)
                dens65s.append(t65)

            def attn_pair(qg, hp, kts, filler=None):
                """score/exp/PV chains for one head pair, then the division
                pipeline; returns the divided+packed ctx pair [128, QG].
                `filler` emits independent PE work between the PV chunks and
                the division ops so the PE stays fed while the chain
                drains."""
                klo = kc_lo[qg] * KC
                kcs = list(range(kc_lo[qg], 4 * (qg + 1)))
                h0, h1 = 2 * hp, 2 * hp + 1
                cps_pair = []
                for h in (h0, h1):
                    cps = cpool.tile([VW, QG], F32, tag="ctx")
                    cps_pair.append(cps)
                for kc in kcs:
                    q_off = max(0, KC * (kc - 4 * qg))
                    co = kc * KC - klo
                    sp2 = spool.tile([P, 2, QG], F32, tag="spsum")
                    for j, h in enumerate((h0, h1)):
                        pb = (h % 2) * HD
                        nc.tensor.matmul(
                            sp2[:, j, q_off:],
                            lhsT=kts[pb:pb + HD, hp, co:co + KC],
                            rhs=qT_sb[pb:pb + HD, hp,
                                      qg * QG + q_off:(qg + 1) * QG],
                            start=True,
                            stop=True,
                        )
                    if kc >= 4 * qg:  # diagonal: mask both heads' bands
                        band = sp2[:, :, q_off:q_off + KC]
                        nc.vector.tensor_tensor(
                            band, band,
                            bm_sb[:, None, :].to_broadcast([P, 2, KC]),
                            mybir.AluOpType.add,
                        )
                    pt = ptp.tile([P, 2, QG], MDT, tag="pt")
                    nc.scalar.activation(
                        pt[:, :, q_off:], sp2[:, :, q_off:],
                        mybir.ActivationFunctionType.Exp,
                    )
                    for j, h in enumerate((h0, h1)):
                        nc.tensor.matmul(
                            cps_pair[j][:, q_off:],
                            lhsT=va_sb[:, kc, h * VW:(h + 1) * VW],
                            rhs=pt[:, j, q_off:],
                            start=(kc == kcs[0]),
                            stop=(kc == kcs[-1]),
                        )
                # ---- division pipeline (dens adds + cxf copies release
                # the PSUM banks the NEXT pair's PVs need -- run them at
                # high priority so they slot in right after the last PV) ----
                dens65 = dens65s[(2 * qg + hp) % 3]
                cxf_pair = []
                # tail ops: bank-releasers run hot; the rest of the chain
                # has a full group of slack except in the last group
                slack = -50 if qg < NQG - 1 else 200
                with tc.high_priority(offset=200):
                    nc.vector.tensor_scalar_add(
                        dens65[0:1, :], cps_pair[0][HD:HD + 1, :],
                        cnt_sb[0:1, h0 * NQG + qg:h0 * NQG + qg + 1])
                    nc.scalar.activation(
                        dens65[HD:HD + 1, :], cps_pair[1][HD:HD + 1, :],
                        mybir.ActivationFunctionType.Identity,
                        bias=cnt_sb[0:1, h1 * NQG + qg:h1 * NQG + qg + 1])
                    for j, h in enumerate((h0, h1)):
                        cxf = ctxp.tile([HD, QG], F32, tag="cxf")
                        if j == 0:
                            nc.scalar.copy(cxf, cps_pair[j][:HD, :])
                        else:
                            nc.vector.tensor_copy(cxf, cps_pair[j][:HD, :])
                        cxf_pair.append(cxf)
                with tc.high_priority(offset=slack):
                    rec65 = densp.tile([VW, QG], F32, tag="rec65")
                    nc.vector.reciprocal_approx_fast(rec65, dens65[:])
                    rec65r = densp.tile([VW, QG], mybir.dt.float32r,
                                        tag="rec65r")
                    nc.scalar.activation(
                        rec65r, rec65, mybir.ActivationFunctionType.Copy)
                # rank-1 partition broadcast: bcs2[p, q] = rec65[64*(p//64), q]
                with tc.high_priority(offset=slack):
                    bpool = cpool if qg == NQG - 1 else ppool
                    btag = "ctx" if qg == NQG - 1 else "pp"
                    bcs2 = bpool.tile([P, QG], F32, tag=btag)
                    nc.tensor.matmul(
                        bcs2,
                        lhsT=sel65r_sb[0:VW, :],
                        rhs=rec65r,
                        start=True,
                        stop=True,
                    )
                    # ctx = (near + distant prefix) * (1/den), packed pair.
                    # In the last group, emit per-ss slices so the final
                    # out-projection unblocks as soon as its slice lands.
                    cp2 = ctxp.tile([P, QG], MDT, tag="cpair")
                    sls = ([slice(ss * P, (ss + 1) * P)
                            for ss in range(QG // P)]
                           if qg == NQG - 1 else [slice(0, QG)])
                    for sl2 in sls:
                        for j, h in enumerate((h0, h1)):
                            nc.vector.scalar_tensor_tensor(
                                cp2[j * HD:(j + 1) * HD, sl2], cxf_pair[j][:, sl2],
                                pft_sb[:HD, qg * NH + h:qg * NH + h + 1],
                                bcs2[j * HD:(j + 1) * HD, sl2],
                                mybir.AluOpType.add,
                                mybir.AluOpType.mult,
                            )
                return cp2

            def outproj(qg, pairs, sss, opss=None):
                for ss in sss:
                    if opss is not None and ss in opss:
                        ops = opss[ss]
                    else:
                        ops = ppool.tile([P, DM], F32, tag="pp")
                    for hp in range(2):
                        if opss is not None and ss in opss and hp == 0:
                            continue
                        nc.tensor.matmul(
                            ops,
                            lhsT=pairs[hp][:, ss * P:(ss + 1) * P],
                            rhs=wo_sb[:, hp, :],
                            start=(hp == 0),
                            stop=(hp == 1),
                        )
                    osb = ptp.tile([P, DM], F32, tag="osb")
                    with tc.high_priority(
                            offset=(200 if qg == NQG - 1 else -100)):
                        if ss % 2 == 0:
                            nc.vector.tensor_copy(osb, ops)
                        else:
                            nc.scalar.copy(osb, ops)
                    deng = nc.sync if ss % 2 == 0 else nc.gpsimd
                    deng.dma_start(
                        out_d[:][qg * QG + ss * P:qg * QG + (ss + 1) * P, :],
                        osb,
                    )

            def outproj_first_pair(pairs0, sss):
                """start=True half-accumulations for the last group's first
                pair so the final tail only runs the stop half."""
                opss = {}
                for ss in sss:
                    ops = ppool.tile([P, DM], F32, tag="pp")
                    nc.tensor.matmul(
                        ops,
                        lhsT=pairs0[:, ss * P:(ss + 1) * P],
                        rhs=wo_sb[:, 0, :],
                        start=True,
                        stop=False,
                    )
                    opss[ss] = ops
                return opss

            # lookahead: q/k projections, b-fold and the NEXT group's
            # projections run ahead of the attention stream consuming them
            qps = proj_qk(0)
            ktss = [prep(0)]
            fold(0, *ktss[0], qps)
            proj_v(0)
            qps = proj_qk(1)
            ktss.append(prep(1))
            fold(1, *ktss[1], qps)
            proj_v(1)
            pending = None
            for qg in range(NQG):
                cp0 = attn_pair(qg, 0, ktss[qg][0])
                # mid-group PE filler: the pair-0 division chain stalls the
                # PE otherwise (its PSUM banks gate the next pair's PVs)
                if qg + 2 < NQG:
                    proj_v(qg + 2)
                elif pending is not None:
                    outproj(*pending, sss=(0, 1, 2, 3))
                    pending = None
                cp1 = attn_pair(qg, 1, ktss[qg][0])
                if qg + 2 < NQG:
                    qps = proj_qk(qg + 2)
                    ktss.append(prep(qg + 2))
                    fold(qg + 2, *ktss[qg + 2], qps)
                if pending is not None:
                    outproj(*pending, sss=(0, 1, 2, 3))
                if qg == NQG - 1:
                    opss = outproj_first_pair(cp0, (0, 1))
                pending = (qg, (cp0, cp1))
            outproj(*pending, sss=(0, 1, 2, 3), opss=opss)

    nc.finalize()
    return nc


# --------------------------------------------------------------------------
# host wrapper
# --------------------------------------------------------------------------

def _is_tril(mask: np.ndarray) -> bool:
    tril = np.tril(np.ones((S, S), dtype=mask.dtype))
    return all(np.array_equal(mask[b], tril) for b in range(mask.shape[0]))


def _prep_core_inputs(x, days, Wq, bq, Wk, bk, Wv, bv, Wo, rate,
                      use_bf16):
    """Per-core in_maps plus static loop bounds (shared across cores)."""
    t = days.astype(np.float64)  # [B, S]
    # distance beyond which |s| * decay < ~3e-4, i.e. exp(s * decay) is 1
    # to within 3e-4 relative (|s| <= 150 is a generous bound).
    d_cut = (np.log(150.0) + np.log(1.0 / 3e-4)) / rate
    # static near-window bounds (min over batches so one program fits all)
    kc_lo = []
    for qg in range(NQG):
        lo = NKC
        for b in range(B):
            tq = t[b, qg * QG]
            c = 0
            while c < 4 * qg and t[b, c * KC + KC - 1] < tq - d_cut:
                c += 1
            lo = min(lo, c)
        kc_lo.append(lo)
    kc_lo = tuple(kc_lo)
    wmax = max((qg + 1) * QG - kc_lo[qg] * KC for qg in range(NQG))
    wmax = ((wmax + P - 1) // P) * P

    # per-batch decay factor vectors (f64 for exactness, then f32)
    scale = 1.0 / np.sqrt(HD)
    t0 = np.stack([(t[:, qg * QG] + t[:, qg * QG + QG - 1]) * 0.5
                   for qg in range(NQG)], axis=1)  # [B, NQG]
    avec = np.zeros((B, 1, S), np.float32)
    bvec = np.zeros((B, NQG, S), np.float32)
    for b in range(B):
        for qg in range(NQG):
            sl = slice(qg * QG, (qg + 1) * QG)
            avec[b, 0, sl] = (np.exp(-rate * (t[b, sl] - t0[b, qg])) * scale
                              ).astype(np.float32)
            hi = (qg + 1) * QG
            bvec[b, qg, :hi] = (np.exp(rate * (t[b, :hi] - t0[b, qg]))
                                ).astype(np.float32)
    assert np.all(np.isfinite(avec)) and np.all(np.isfinite(bvec)), \
        "decay factor overflow; q-group span too large for fast path"

    # additive causal mask for the diagonal block: 0 iff q >= k else -1e30
    kl = np.arange(P)[:, None]
    ql = np.arange(P)[None, :]
    bandm = np.where(ql >= kl, 0.0, NEG).astype(np.float32)
    # sel65: broadcast selector -- rec row 0 to partitions 0-63, rec row
    # 64 to partitions 64-127
    sel65 = np.zeros((P, P), np.float32)
    sel65[0, 0:HD] = 1.0
    sel65[HD, HD:P] = 1.0

    with_bqk = bool(np.any(bq != 0) or np.any(bk != 0))
    with_bv = bool(np.any(bv != 0))

    in_maps = []
    for c in range(NCORES):
        b, hg = divmod(c, NHG)
        cols = slice(hg * HGD, (hg + 1) * HGD)
        # prefix V sums for the distant rank-1 update: [HD, NQG*NH] (hd-major)
        prefv = np.zeros((HD, NQG * NH), np.float32)
        cnt = np.zeros((NH, NQG), np.float32)
        for qg in range(NQG):
            n = kc_lo[qg] * KC
            cnt[:, qg] = float(n)
            if n > 0:
                xs = x[b, :n].astype(np.float64).sum(axis=0)  # [DM]
                vs = xs @ Wv[cols, :].astype(np.float64).T \
                    + n * bv[cols].astype(np.float64)
                for h in range(NH):
                    prefv[:, qg * NH + h] = \
                        vs[h * HD:(h + 1) * HD].astype(np.float32)
        mdt = np.dtype(ml_dtypes.bfloat16) if use_bf16 else np.float32
        # packed const block: (unused) | pft | cnt | band mask
        cpk = np.zeros((P, 288), np.float32)
        cpk[0:HD, P:P + 16] = prefv
        cpk[0, P + 16:P + 32] = cnt.reshape(-1)
        cpk[:, P + 32:P + 160] = bandm
        def dev3(a, inner):  # [(ko p), m] -> [P, ko*m] device layout
            ko = a.shape[0] // P
            return np.ascontiguousarray(
                a.reshape(ko, P, inner).transpose(1, 0, 2).reshape(P, -1))
        m = {
            "xT": dev3(x[b].T, S).astype(mdt),
            "wqT": dev3(Wq[cols, :].T, HGD).astype(mdt),
            "wkT": dev3(Wk[cols, :].T, HGD).astype(mdt),
            "wvT": dev3(Wv[cols, :].T, HGD).astype(mdt),
            "woT": dev3(Wo[:, cols].T, DM).astype(mdt),
            "avec": avec[b],
            "bvec": bvec[b],
            "cpk": cpk,
            "sel65r": sel65,
        }
        if with_bqk:
            m["bq"] = np.ascontiguousarray(
                bq[cols].reshape(2, P).T).astype(np.float32)
            m["bk"] = np.ascontiguousarray(
                bk[cols].reshape(2, P).T).astype(np.float32)
        if with_bv:
            m["bvb"] = bv[cols].reshape(1, HGD).astype(np.float32)
        in_maps.append(m)
    return in_maps, kc_lo, wmax, with_bqk, with_bv


def _reference_host(x, mask, days_offset, Wq, bq, Wk, bk, Wv, bv, Wo, bo,
                    decay_rate):
    """Emergency numpy fallback for inputs outside the fast path."""
    b, s, _ = x.shape
    out = np.empty((b, s, DM), np.float32)
    for bi in range(b):
        q = (x[bi] @ Wq.T + bq).reshape(s, H, HD).transpose(1, 0, 2)
        k = (x[bi] @ Wk.T + bk).reshape(s, H, HD).transpose(1, 0, 2)
        v = (x[bi] @ Wv.T + bv).reshape(s, H, HD).transpose(1, 0, 2)
        dist = np.abs(days_offset[bi][:, None] - days_offset[bi][None, :])
        decay = np.exp(-decay_rate * dist).astype(np.float32)
        ctx = np.empty((H, s, HD), np.float32)
        for h in range(H):
            sc = (q[h] @ k[h].T) / np.sqrt(HD) * decay
            sc = np.where(mask[bi] == 0, -np.inf, sc)
            sc = sc - sc.max(axis=-1, keepdims=True)
            e = np.exp(sc)
            ctx[h] = (e / e.sum(axis=-1, keepdims=True)) @ v[h]
        out[bi] = ctx.transpose(1, 0, 2).reshape(s, DM) @ Wo.T + bo
    return out


def kernel(x, mask, days_offset, Wq, bq, Wk, bk, Wv, bv, Wo, bo, decay_rate,
           _trace=False):
    x = np.asarray(x, np.float32)
    mask = np.asarray(mask)
    days = np.asarray(days_offset, np.float32)
    Wq, bq = np.asarray(Wq, np.float32), np.asarray(bq, np.float32)
    Wk, bk = np.asarray(Wk, np.float32), np.asarray(bk, np.float32)
    Wv, bv = np.asarray(Wv, np.float32), np.asarray(bv, np.float32)
    Wo, bo = np.asarray(Wo, np.float32), np.asarray(bo, np.float32)
    rate = float(np.asarray(decay_rate))

    sorted_ok = bool(np.all(np.diff(days, axis=-1) >= 0))
    if not (sorted_ok and _is_tril(mask)):
        return _reference_host(x, mask, days, Wq, bq, Wk, bk, Wv, bv, Wo, bo,
                               rate)

    use_bf16 = os.environ.get("KERNEL_F32", "") != "1"
    in_maps, kc_lo, wmax, with_bqk, with_bv = _prep_core_inputs(
        x, days, Wq, bq, Wk, bk, Wv, bv, Wo, rate, use_bf16)

    key = (kc_lo, wmax, with_bqk, with_bv, use_bf16)
    if key not in _cache:
        _cache[key] = _build_fast(kc_lo, wmax, with_bqk, with_bv, use_bf16)
    nc = _cache[key]

    res = run_bass_kernel_spmd(nc, in_maps, core_ids=list(range(NCORES)),
                               trace=_trace)
    out = np.empty((B, S, DM), np.float32)
    for b in range(B):
        out[b] = res.results[2 * b]["outp"] + res.results[2 * b + 1]["outp"] + bo
    if _trace:
        return out, res
    return out
